# revision 32
# baseline (speedup 1.0000x reference)
"""BPR embedding-lookup kernel for 8 TRN2 NeuronCores.

Math (per batch element b):
    out[b] = dot(user_emb[users[b]], item_emb[items[b]])
           + sum_u social_weight[users[b], u] * dot(item_emb[items[b]], user_emb[u])

Reformulated per element as a single 64-length dot:
    out[b] = sum_d biT[d,b] * (V[d,b] + buT[d,b]),
    V[:,b] = user_emb.T @ social_weight[users[b], :]     (PE-accumulated)

Sharding: sort batch by user index, split into 8 contiguous chunks of 512.
Core m receives the contiguous social_weight row range covering its chunk's
users (~1/8 of the table) so row gathers stay local; output is
inverse-permuted on the host.

Per-core device pipeline:
  - social_weight rows arrive TRANSPOSED in SBUF via gpsimd dma_gather
    (transpose=True, bf16): partition = u%128, free = (u//128, b).
    This feeds the PE directly: 79 accumulating matmuls per 128-batch block
    with user_emb k-chunks stationary produce V^T [64, 512] in PSUM —
    no elementwise multiply or reduction pass over the [B,U] block at all.
  - bu rows arrive transposed the same way; bi rows (item ids exceed int16)
    use indirect DMA + DVE 32x32 block transposes.
  - tail: tmp = biT * (V^T + buT) on DVE, ones-matmul folds the d-reduction
    (pos + social together), ACT copies PSUM out.
"""

import sys

if "/opt/trn_rl_repo" not in sys.path:
    sys.path.insert(0, "/opt/trn_rl_repo")

import numpy as np

NUM_USERS = 10000
NUM_ITEMS = 100000
D = 64
B = 4096
NCORES = 8
BL = B // NCORES          # 512 batch elements per core
UK = 10112                # num_users padded to 79*128 (dma_gather needs 256B elems)
KC = UK // 128            # 79 contraction chunks
NG = 4                    # item-gather blocks per core (128 indices each)
GB = BL // NG
# k-slices for the social_weight transpose-gathers; elem bytes must be %256.
# Descending sizes: the last slice is small so the post-DMA matmul tail
# (run at cold PE clock) is short.
KSLICES = [(0, 3328), (3328, 3328), (6656, 2560), (9216, 896)]

_PROGRAM_CACHE = {}
LAST_RESULTS = None


def _build_program(s_pad: int):
    import ml_dtypes  # noqa: F401

    from concourse import bacc, bass, mybir, tile

    f32 = mybir.dt.float32
    bf16 = mybir.dt.bfloat16
    i16 = mybir.dt.int16
    i32 = mybir.dt.int32
    mult = mybir.AluOpType.mult
    add = mybir.AluOpType.add

    nc = bacc.Bacc(
        "TRN2",
        target_bir_lowering=False,
        debug=False,
        num_devices=NCORES,
        num_swdge_queues=4,
    )
    swp_d = nc.declare_dram_parameter("swp", [s_pad, UK], bf16, isOutput=False)
    # pre-arranged on host: uembk[p, c*D + d] = user_emb_padded[c*128 + p, d]
    uembk_d = nc.declare_dram_parameter("uembk", [128, KC * D], bf16, isOutput=False)
    uemb128_d = nc.declare_dram_parameter("uemb128", [NUM_USERS, 128], bf16, isOutput=False)
    iemb_d = nc.declare_dram_parameter("iemb", [NUM_ITEMS, D], f32, isOutput=False)
    swidx_d = nc.declare_dram_parameter("swidx", [128, BL // 16], i16, isOutput=False)
    ugidx16_d = nc.declare_dram_parameter("ugidx16", [128, BL // 16], i16, isOutput=False)
    iidx_d = nc.declare_dram_parameter("iidx", [128, NG], i32, isOutput=False)
    out_d = nc.declare_dram_parameter("out", [1, BL], f32, isOutput=True)

    with tile.TileContext(nc) as tc:
        with (
            tc.tile_pool(name="const", bufs=1) as constp,
            tc.tile_pool(name="swt", bufs=1) as swtp,
            tc.tile_pool(name="small", bufs=4) as smallp,
            tc.tile_pool(name="psum", bufs=2, space="PSUM") as psump,
            tc.tile_pool(name="psum2", bufs=2, space="PSUM") as psum2p,
        ):
            swidx_t = constp.tile([128, BL // 16], i16)
            nc.sync.dma_start(out=swidx_t[:], in_=swidx_d[:])
            ugidx_t = constp.tile([128, BL // 16], i16)
            nc.sync.dma_start(out=ugidx_t[:], in_=ugidx16_d[:])
            iidx_t = constp.tile([128, NG], i32)
            nc.sync.dma_start(out=iidx_t[:], in_=iidx_d[:])

            # social_weight transposed gathers: each covers ALL 512 batch
            # columns for a contiguous k-slice of the row, so matmuls are
            # 512 wide and the post-DMA PE tail is only the last slice.
            # Small item-row gathers are interleaved so none gets stranded
            # behind the whole gather train.
            biT = constp.tile([D, BL], f32)
            bis = []
            swts = []
            for g, (koff, ksz) in enumerate(KSLICES):
                swt = swtp.tile([128, ksz // 128, BL], bf16, tag=f"swt{g}")
                nc.gpsimd.dma_gather(
                    out_ap=swt[:],
                    in_ap=swp_d[:, koff : koff + ksz],
                    idxs_ap=swidx_t[:],
                    num_idxs=BL,
                    num_idxs_reg=BL,
                    elem_size=ksz,
                    elem_step=UK,
                    transpose=True,
                    queue_num=g % 4,
                )
                swts.append(swt)
                if g >= 1:
                    # two small item gathers between the big desc-gens
                    for _ in range(2):
                        if len(bis) < NG:
                            bi = smallp.tile([128, D], f32, tag="bi")
                            nc.gpsimd.indirect_dma_start(
                                out=bi[:],
                                out_offset=None,
                                in_=iemb_d[:],
                                in_offset=bass.IndirectOffsetOnAxis(
                                    ap=iidx_t[:, len(bis) : len(bis) + 1], axis=0
                                ),
                            )
                            bis.append(bi)

            # buT[d, b] = user_emb[users[b], d] (partitions 64.. zero-padded)
            buT = constp.tile([128, 1, BL], bf16)
            nc.gpsimd.dma_gather(
                out_ap=buT[:],
                in_ap=uemb128_d[:],
                idxs_ap=ugidx_t[:],
                num_idxs=BL,
                num_idxs_reg=BL,
                elem_size=128,
                transpose=True,
            )

            uembk_t = constp.tile([128, KC, D], bf16)
            nc.sync.dma_start(
                out=uembk_t[:], in_=uembk_d[:].rearrange("p (c d) -> p c d", d=D)
            )
            ones_t = constp.tile([D, 1], f32)
            nc.vector.memset(ones_t[:], 1.0)
            ident = constp.tile([128, 128], f32)
            from concourse.masks import make_identity

            make_identity(nc, ident[:])

            # biT[d, b] = item_emb[items[b], d] via PE transpose (PE is idle
            # by the time the gather-completion lanes release the bi tiles)
            for g in range(NG):
                bi_ps = psum2p.tile([D, 128], f32, tag="bits")
                nc.tensor.transpose(out=bi_ps[:], in_=bis[g][:], identity=ident[:])
                nc.scalar.copy(out=biT[:, g * 128 : (g + 1) * 128], in_=bi_ps[:])

            # V^T[d, b] accumulated over all 79 k-chunks (512-wide matmuls).
            # Two interleaved PSUM accumulation chains so LDWEIGHTS of one
            # chain pipelines under the MATMUL of the other.
            vt_ps0 = psump.tile([D, BL], f32, tag="vt0")
            vt_ps1 = psump.tile([D, BL], f32, tag="vt1")
            chains = [vt_ps0, vt_ps1]
            for g, (koff, ksz) in enumerate(KSLICES):
                for c in range(ksz // 128):
                    kchunk = koff // 128 + c
                    par = kchunk % 2
                    nc.tensor.matmul(
                        out=chains[par][:],
                        lhsT=uembk_t[:, kchunk, :],
                        rhs=swts[g][:, c, :],
                        start=(kchunk < 2),
                        stop=(kchunk >= KC - 2),
                    )

            vt_sum = constp.tile([D, BL], f32)
            nc.vector.tensor_tensor(
                out=vt_sum[:], in0=vt_ps0[:], in1=buT[:D, 0, :], op=add
            )
            tmp2 = constp.tile([D, BL], f32)
            nc.vector.tensor_tensor(out=tmp2[:], in0=vt_sum[:], in1=vt_ps1[:], op=add)
            tmp3 = constp.tile([D, BL], f32)
            nc.vector.tensor_tensor(out=tmp3[:], in0=tmp2[:], in1=biT[:], op=mult)
            res_ps = psum2p.tile([1, BL], f32)
            nc.tensor.matmul(
                out=res_ps[:], lhsT=ones_t[:], rhs=tmp3[:], start=True, stop=True
            )
            res_t = constp.tile([1, BL], f32)
            nc.scalar.copy(out=res_t[:], in_=res_ps[:])
            nc.sync.dma_start(out=out_d[:], in_=res_t[:])

    nc.finalize()
    return nc


def _wrap16(idx):
    """[BL] int -> [128, BL//16] int16: idx i at (i%16, i//16), replicated x8."""
    n = len(idx)
    blk = np.empty((16, n // 16), np.int16)
    blk[np.arange(n) % 16, np.arange(n) // 16] = idx.astype(np.int16)
    return np.ascontiguousarray(np.tile(blk, (8, 1)))


def kernel(user_emb, item_emb, social_weight, users, items):
    global LAST_RESULTS
    import os

    import ml_dtypes

    from concourse.bass_utils import run_bass_kernel_spmd

    bf = ml_dtypes.bfloat16
    user_emb = np.ascontiguousarray(np.asarray(user_emb, dtype=np.float32))
    item_emb = np.ascontiguousarray(np.asarray(item_emb, dtype=np.float32))
    social_weight = np.ascontiguousarray(np.asarray(social_weight, dtype=np.float32))
    users = np.asarray(users).astype(np.int64)
    items = np.asarray(items).astype(np.int64)

    order = np.argsort(users, kind="stable")
    users_s = users[order]
    items_s = items[order]

    los, spans = [], []
    for m in range(NCORES):
        seg = users_s[m * BL : (m + 1) * BL]
        lo = int(seg[0])
        hi = int(seg[-1]) + 1
        los.append(lo)
        spans.append(hi - lo)
    s_pad = max(spans)

    if s_pad not in _PROGRAM_CACHE:
        _PROGRAM_CACHE[s_pad] = _build_program(s_pad)
    nc = _PROGRAM_CACHE[s_pad]

    uembk_pad = np.zeros((UK, D), bf)
    uembk_pad[:NUM_USERS] = user_emb.astype(bf)
    # [128, KC*D] with uembk[p, c*D+d] = uemb_pad[c*128+p, d]
    uembk = np.ascontiguousarray(
        uembk_pad.reshape(KC, 128, D).transpose(1, 0, 2).reshape(128, KC * D)
    )
    uemb128 = np.zeros((NUM_USERS, 128), bf)
    uemb128[:, :D] = user_emb.astype(bf)

    in_maps = []
    for m in range(NCORES):
        seg_ug = users_s[m * BL : (m + 1) * BL]
        seg_u = (seg_ug - los[m]).astype(np.int64)
        seg_i = items_s[m * BL : (m + 1) * BL].astype(np.int32)
        swp = np.zeros((s_pad, UK), bf)
        swp[: spans[m], :NUM_USERS] = social_weight[los[m] : los[m] + spans[m]].astype(
            bf
        )
        in_maps.append(
            {
                "swp": swp,
                "uembk": uembk,
                "uemb128": uemb128,
                "iemb": item_emb,
                "swidx": _wrap16(seg_u),
                "ugidx16": _wrap16(seg_ug),
                "iidx": np.ascontiguousarray(seg_i.reshape(NG, 128).T),
            }
        )

    trace = bool(os.environ.get("CC_KERNEL_TRACE"))
    tmpdir = os.environ.get("CC_TRACE_DIR") or None
    res = run_bass_kernel_spmd(
        nc, in_maps, list(range(NCORES)), trace=trace, tmpdir=tmpdir
    )
    LAST_RESULTS = res

    out_sorted = np.empty(B, np.float32)
    for m in range(NCORES):
        out_sorted[m * BL : (m + 1) * BL] = np.asarray(res.results[m]["out"])[0]

    final = np.empty(B, np.float32)
    final[order] = out_sorted
    return final


# revision 35
# speedup vs baseline: 1.3008x; 1.3008x over previous
"""BPR embedding-lookup kernel for 8 TRN2 NeuronCores.

Math (per batch element b):
    out[b] = dot(user_emb[users[b]], item_emb[items[b]])
           + sum_u social_weight[users[b], u] * dot(item_emb[items[b]], user_emb[u])

Reformulated per element as a single 64-length dot:
    out[b] = sum_d biT[d,b] * (V[d,b] + buT[d,b]),
    V[:,b] = user_emb.T @ social_weight[users[b], :]     (PE-accumulated)

Sharding: sort batch by user index, split into 8 contiguous chunks of 512.
Core m receives the contiguous social_weight row range covering its chunk's
users (~1/8 of the table) so row gathers stay local; output is
inverse-permuted on the host.

Per-core device pipeline:
  - social_weight rows arrive TRANSPOSED in SBUF via gpsimd dma_gather
    (transpose=True, bf16): partition = u%128, free = (u//128, b).
    This feeds the PE directly: 79 accumulating matmuls per 128-batch block
    with user_emb k-chunks stationary produce V^T [64, 512] in PSUM —
    no elementwise multiply or reduction pass over the [B,U] block at all.
  - bu rows arrive transposed the same way; bi rows (item ids exceed int16)
    use indirect DMA + DVE 32x32 block transposes.
  - tail: tmp = biT * (V^T + buT) on DVE, ones-matmul folds the d-reduction
    (pos + social together), ACT copies PSUM out.
"""

import sys

if "/opt/trn_rl_repo" not in sys.path:
    sys.path.insert(0, "/opt/trn_rl_repo")

import numpy as np

NUM_USERS = 10000
NUM_ITEMS = 100000
D = 64
B = 4096
NCORES = 8
BL = B // NCORES          # 512 batch elements per core
UK = 10112                # num_users padded to 79*128 (dma_gather needs 256B elems)
KC = UK // 128            # 79 contraction chunks
NG = 4                    # item-gather blocks per core (128 indices each)
GB = BL // NG
# k-slices for the social_weight transpose-gathers; elem bytes must be %256.
# Descending sizes: the last slice is small so the post-DMA matmul tail
# (run at cold PE clock) is short.
KSLICES = [(0, 3328), (3328, 3328), (6656, 2560), (9216, 896)]

_PROGRAM_CACHE = {}
LAST_RESULTS = None


def _build_program(s_pad: int):
    import ml_dtypes  # noqa: F401

    from concourse import bacc, bass, mybir, tile

    f32 = mybir.dt.float32
    bf16 = mybir.dt.bfloat16
    i16 = mybir.dt.int16
    i32 = mybir.dt.int32
    mult = mybir.AluOpType.mult
    add = mybir.AluOpType.add

    nc = bacc.Bacc(
        "TRN2",
        target_bir_lowering=False,
        debug=False,
        num_devices=NCORES,
        num_swdge_queues=4,
    )
    swp_d = nc.declare_dram_parameter("swp", [s_pad, UK], bf16, isOutput=False)
    # pre-arranged on host: uembk[p, c*D + d] = user_emb_padded[c*128 + p, d]
    uembk_d = nc.declare_dram_parameter("uembk", [128, KC * D], bf16, isOutput=False)
    uemb128_d = nc.declare_dram_parameter("uemb128", [NUM_USERS, 128], bf16, isOutput=False)
    iemb_d = nc.declare_dram_parameter("iemb", [NUM_ITEMS, D], f32, isOutput=False)
    swidx_d = nc.declare_dram_parameter("swidx", [128, BL // 16], i16, isOutput=False)
    ugidx16_d = nc.declare_dram_parameter("ugidx16", [128, BL // 16], i16, isOutput=False)
    iidx_d = nc.declare_dram_parameter("iidx", [128, NG], i32, isOutput=False)
    out_d = nc.declare_dram_parameter("out", [128, NG], f32, isOutput=True)

    with tile.TileContext(nc) as tc:
        with (
            tc.tile_pool(name="const", bufs=1) as constp,
            tc.tile_pool(name="swt", bufs=1) as swtp,
            tc.tile_pool(name="small", bufs=4) as smallp,
            tc.tile_pool(name="psum", bufs=2, space="PSUM") as psump,
            tc.tile_pool(name="psum2", bufs=2, space="PSUM") as psum2p,
        ):
            swidx_t = constp.tile([128, BL // 16], i16)
            nc.sync.dma_start(out=swidx_t[:], in_=swidx_d[:])
            ugidx_t = constp.tile([128, BL // 16], i16)
            nc.sync.dma_start(out=ugidx_t[:], in_=ugidx16_d[:])
            iidx_t = constp.tile([128, NG], i32)
            nc.sync.dma_start(out=iidx_t[:], in_=iidx_d[:])

            # social_weight transposed gathers: each covers ALL 512 batch
            # columns for a contiguous k-slice of the row, so matmuls are
            # 512 wide and the post-DMA PE tail is only the last slice.
            # Small item-row gathers are interleaved so none gets stranded
            # behind the whole gather train.
            bis = []
            swts = []
            for g, (koff, ksz) in enumerate(KSLICES):
                swt = swtp.tile([128, ksz // 128, BL], bf16, tag=f"swt{g}")
                nc.gpsimd.dma_gather(
                    out_ap=swt[:],
                    in_ap=swp_d[:, koff : koff + ksz],
                    idxs_ap=swidx_t[:],
                    num_idxs=BL,
                    num_idxs_reg=BL,
                    elem_size=ksz,
                    elem_step=UK,
                    transpose=True,
                    queue_num=g % 4,
                )
                swts.append(swt)
                if g >= 1:
                    # two small item gathers between the big desc-gens
                    for _ in range(2):
                        if len(bis) < NG:
                            bi = smallp.tile([128, D], f32, tag="bi")
                            nc.gpsimd.indirect_dma_start(
                                out=bi[:],
                                out_offset=None,
                                in_=iemb_d[:],
                                in_offset=bass.IndirectOffsetOnAxis(
                                    ap=iidx_t[:, len(bis) : len(bis) + 1], axis=0
                                ),
                            )
                            bis.append(bi)

            # buT[d, b] = user_emb[users[b], d] (partitions 64.. zero-padded)
            buT = constp.tile([128, 1, BL], bf16)
            nc.gpsimd.dma_gather(
                out_ap=buT[:],
                in_ap=uemb128_d[:],
                idxs_ap=ugidx_t[:],
                num_idxs=BL,
                num_idxs_reg=BL,
                elem_size=128,
                transpose=True,
            )

            uembk_t = constp.tile([128, KC, D], bf16)
            nc.sync.dma_start(
                out=uembk_t[:], in_=uembk_d[:].rearrange("p (c d) -> p c d", d=D)
            )
            ident = constp.tile([D, D], f32)
            from concourse.masks import make_identity

            make_identity(nc, ident[:])

            # V^T[d, b] accumulated over all 79 k-chunks (512-wide matmuls).
            # Two interleaved PSUM accumulation chains so LDWEIGHTS of one
            # chain pipelines under the MATMUL of the other.
            vt_ps0 = psump.tile([D, BL], f32, tag="vt0")
            vt_ps1 = psump.tile([D, BL], f32, tag="vt1")
            chains = [vt_ps0, vt_ps1]
            for g, (koff, ksz) in enumerate(KSLICES):
                for c in range(ksz // 128):
                    kchunk = koff // 128 + c
                    par = kchunk % 2
                    nc.tensor.matmul(
                        out=chains[par][:],
                        lhsT=uembk_t[:, kchunk, :],
                        rhs=swts[g][:, c, :],
                        start=(kchunk < 2),
                        stop=(kchunk >= KC - 2),
                    )

            vt_sum = constp.tile([D, BL], f32)
            nc.vector.tensor_tensor(
                out=vt_sum[:], in0=vt_ps0[:], in1=buT[:D, 0, :], op=add
            )
            tmp2 = constp.tile([D, BL], f32)
            nc.vector.tensor_tensor(out=tmp2[:], in0=vt_sum[:], in1=vt_ps1[:], op=add)
            # Per 128-batch block: PE-transpose tmp2 (after all matmuls, so no
            # queue stall), multiply by natural-layout bi on DVE, row-reduce.
            out_stage = constp.tile([128, NG], f32)
            for g in range(NG):
                t_ps = psum2p.tile([128, D], f32, tag="tps")
                nc.tensor.transpose(
                    out=t_ps[:],
                    in_=tmp2[:, g * 128 : (g + 1) * 128],
                    identity=ident[:],
                )
                t_sb = smallp.tile([128, D], f32, tag="tsb")
                nc.scalar.copy(out=t_sb[:], in_=t_ps[:])
                prod = smallp.tile([128, D], f32, tag="prod")
                nc.vector.tensor_tensor(
                    out=prod[:], in0=bis[g][:], in1=t_sb[:], op=mult
                )
                nc.vector.tensor_reduce(
                    out=out_stage[:, g : g + 1],
                    in_=prod[:],
                    axis=mybir.AxisListType.X,
                    op=add,
                )
            nc.sync.dma_start(out=out_d[:], in_=out_stage[:])

    nc.finalize()
    return nc


def _wrap16(idx):
    """[BL] int -> [128, BL//16] int16: idx i at (i%16, i//16), replicated x8."""
    n = len(idx)
    blk = np.empty((16, n // 16), np.int16)
    blk[np.arange(n) % 16, np.arange(n) // 16] = idx.astype(np.int16)
    return np.ascontiguousarray(np.tile(blk, (8, 1)))


def kernel(user_emb, item_emb, social_weight, users, items):
    global LAST_RESULTS
    import os

    import ml_dtypes

    from concourse.bass_utils import run_bass_kernel_spmd

    bf = ml_dtypes.bfloat16
    user_emb = np.ascontiguousarray(np.asarray(user_emb, dtype=np.float32))
    item_emb = np.ascontiguousarray(np.asarray(item_emb, dtype=np.float32))
    social_weight = np.ascontiguousarray(np.asarray(social_weight, dtype=np.float32))
    users = np.asarray(users).astype(np.int64)
    items = np.asarray(items).astype(np.int64)

    order = np.argsort(users, kind="stable")
    users_s = users[order]
    items_s = items[order]

    los, spans = [], []
    for m in range(NCORES):
        seg = users_s[m * BL : (m + 1) * BL]
        lo = int(seg[0])
        hi = int(seg[-1]) + 1
        los.append(lo)
        spans.append(hi - lo)
    s_pad = max(spans)

    if s_pad not in _PROGRAM_CACHE:
        _PROGRAM_CACHE[s_pad] = _build_program(s_pad)
    nc = _PROGRAM_CACHE[s_pad]

    uembk_pad = np.zeros((UK, D), bf)
    uembk_pad[:NUM_USERS] = user_emb.astype(bf)
    # [128, KC*D] with uembk[p, c*D+d] = uemb_pad[c*128+p, d]
    uembk = np.ascontiguousarray(
        uembk_pad.reshape(KC, 128, D).transpose(1, 0, 2).reshape(128, KC * D)
    )
    uemb128 = np.zeros((NUM_USERS, 128), bf)
    uemb128[:, :D] = user_emb.astype(bf)

    in_maps = []
    for m in range(NCORES):
        seg_ug = users_s[m * BL : (m + 1) * BL]
        seg_u = (seg_ug - los[m]).astype(np.int64)
        seg_i = items_s[m * BL : (m + 1) * BL].astype(np.int32)
        swp = np.zeros((s_pad, UK), bf)
        swp[: spans[m], :NUM_USERS] = social_weight[los[m] : los[m] + spans[m]].astype(
            bf
        )
        in_maps.append(
            {
                "swp": swp,
                "uembk": uembk,
                "uemb128": uemb128,
                "iemb": item_emb,
                "swidx": _wrap16(seg_u),
                "ugidx16": _wrap16(seg_ug),
                "iidx": np.ascontiguousarray(seg_i.reshape(NG, 128).T),
            }
        )

    trace = bool(os.environ.get("CC_KERNEL_TRACE"))
    tmpdir = os.environ.get("CC_TRACE_DIR") or None
    res = run_bass_kernel_spmd(
        nc, in_maps, list(range(NCORES)), trace=trace, tmpdir=tmpdir
    )
    LAST_RESULTS = res

    out_sorted = np.empty(B, np.float32)
    for m in range(NCORES):
        o = np.asarray(res.results[m]["out"])  # [128, NG]
        out_sorted[m * BL : (m + 1) * BL] = o.T.reshape(-1)

    final = np.empty(B, np.float32)
    final[order] = out_sorted
    return final
